# revision 43
# baseline (speedup 1.0000x reference)
"""Grouped-expert SwiGLU (MoE) kernel for Trainium2, expert-parallel over 8 cores.

Per core (one expert):
    g = x @ W_gate          [T, DOUT]
    u = x @ W_down          [T, DOUT]
    h = silu(g) * u
    out = h @ W_up          [T, DIN]

All inputs are pre-cast to bf16 and pre-laid-out on the host so the device
does no transposes and no input casts — the PE runs a dense LDW+MM stream at
the bf16 roofline (~216 ns per [128x128]x[128x512] matmul):
  x_t    [S1, P, KC*NS]   strip-contiguous xT: x_t[s,p,k*NS+n] = x[s*NS+n, k*P+p]
  gate_t [JC, P, DIN]     per-j panels: gate_t[j,p,k*P+n] = Wg[k*P+p, j*P+n]
  down_t [JC, P, DIN]     same layout as gate_t
  up_t   [JC, P, DIN]     up_t[j,p,c] = Wu[j*P+p, c]
phase 1: hT[j] = silu(Wg[:,j].T @ xT) * (Wd[:,j].T @ xT)   [dout, tokens]
phase 2: out[m,:] = sum_j hT[j][:,m].T @ Wu[j,:]           [tokens, din]
Matmuls in bf16 with fp32 PSUM accumulation; out stored bf16 (rel err
~0.0047 vs the fp32 reference) and upcast to fp32 on the host.

DMA notes (measured): each DMA_DIRECT2D dispatch costs ~650ns serialized on
its ring (Sync/Scalar are the two HWDGE rings); a single in-flight transfer
streams ~100GB/s, concurrent transfers ~400GB/s aggregate; the tile
framework cycles ~10 global in-flight DMA semaphore slots, with each
dispatch gated on the completion of its slot's previous transfer.  So the
j0 window emits DMAs in exact consumption order, finest pieces first,
across both rings; later strips use big transfers (fewer dispatches).  The
PE clock ramps 0.65->1.2->2.4GHz over ~3-6us of continuous execution and
drops back on multi-us idle gaps, so WARMUP_MM full-width dummy matmuls
ramp it while the first DMAs are in flight.
"""

import numpy as np
import ml_dtypes

import concourse.bacc as bacc
import concourse.mybir as mybir
from concourse.tile import TileContext
from concourse.bass_utils import run_bass_kernel_spmd

F32 = mybir.dt.float32
BF16 = mybir.dt.bfloat16
SILU = mybir.ActivationFunctionType.Silu
SIGMOID = mybir.ActivationFunctionType.Sigmoid
COPY = mybir.ActivationFunctionType.Copy

E = 8
T, DIN, DOUT = 2048, 2048, 1408
P = 128
NS = 512
KC = DIN // P   # 16 contraction chunks (din)
JC = DOUT // P  # 11 dout blocks
MC = T // P     # 16 token blocks
S1 = T // NS    # 4 token strips
S2 = DIN // NS  # 4 din strips

WARMUP_MM = 9   # dummy matmuls to ramp the PE clock before real data lands
WARMUP_N = 512  # full-width: narrow warmups are LDW-bound (PE array half
                # idle between them) and the DVFS ramp stalls at mid clock


def build_program(sim_safe=False):
    nc = bacc.Bacc(target_bir_lowering=False, trn_type="TRN2")
    # strip-contiguous x layout: x_t[s, p, k*NS+n] = x[s*NS+n, k*P+p], so a
    # whole strip is ONE 2D transfer ([128 x 16KB] = 2MB, 16KB descriptors)
    # and any k-range piece is still contiguous.  Bigger transfers matter
    # because the tile framework cycles only ~10 global in-flight DMA
    # semaphore slots — each slot must carry as many bytes as possible.
    xt = nc.dram_tensor("x_t", [S1, P, KC * NS], BF16,
                        kind="ExternalInput")
    wg = nc.dram_tensor("gate_t", [JC, P, DIN], BF16, kind="ExternalInput")
    wd = nc.dram_tensor("down_t", [JC, P, DIN], BF16, kind="ExternalInput")
    wu = nc.dram_tensor("up_t", [JC, P, DIN], BF16, kind="ExternalInput")
    # out stored bf16 (upcast on host): halves out-DMA bytes and the tail
    out = nc.dram_tensor("out", [T, DIN], BF16, kind="ExternalOutput")

    with TileContext(nc) as tc:
        with tc.tile_pool(name="persist", bufs=1) as persist:
            xts = [persist.tile([P, KC, NS], BF16, tag=f"xts{s}",
                                name=f"xts{s}")
                   for s in range(S1)]
            hT = [persist.tile([P, T], BF16, tag=f"hT{j}", name=f"hT{j}")
                  for j in range(JC)]
            wub = [persist.tile([P, DIN], BF16, tag=f"wub{j}", name=f"wub{j}")
                   for j in range(JC)]
            scratch = persist.tile([P, NS], BF16, tag="scratch",
                                   name="scratch")

            with tc.tile_pool(name="wstage", bufs=2) as wstage, \
                 tc.tile_pool(name="silu", bufs=3) as silu_pool, \
                 tc.tile_pool(name="ostage", bufs=6) as ostage, \
                 tc.tile_pool(name="p1", bufs=2, space="PSUM") as p1, \
                 tc.tile_pool(name="p2", bufs=4, space="PSUM") as p2:

                def x_rhs(s, k):
                    return xts[s][:, k, :]

                def xpiece(ring, s, k0, k1):
                    # k-chunks [k0, k1) of strip s as one 2D transfer
                    ring.dma_start(
                        out=xts[s][:, k0:k1, :],
                        in_=xt.ap()[s][:, k0 * NS:k1 * NS]
                        .rearrange("p (k n) -> p k n", n=NS))

                # PE clock warmup: the PE clock ramps 0.65->1.2->2.4GHz
                # over ~3us of continuous execution, so dummy matmuls keep
                # it busy (and ramping) from the end of the engine barrier
                # (~7us) until real data lands (~10.3us); the first real
                # matmul then runs at full clock.  GpSimd memsets scratch —
                # its queue is free right after the framework's own const
                # memsets, the earliest any engine can write SBUF.  The
                # products are discarded (PSUM slot borrowed from p2's
                # rotation, overwritten with start=True in phase 2).
                nc.gpsimd.memset(scratch[:, :], 0.0)
                pw = p2.tile([P, NS], F32, tag="po", name="pw")
                for w in range(WARMUP_MM):
                    nc.tensor.matmul(pw[:, :WARMUP_N],
                                     lhsT=scratch[:, :P],
                                     rhs=scratch[:, :WARMUP_N],
                                     start=True, stop=True)

                # ---- phase 1: hT[j] = silu(gT) * uT ----
                for j in range(JC):
                    wgp = wstage.tile([P, DIN], BF16, tag="wgp", name=f"wgp{j}")
                    wdp = wstage.tile([P, DIN], BF16, tag="wdp", name=f"wdp{j}")
                    if j == 0:
                        # j0 panels in pieces on the Sync ring, wg/wd
                        # interleaved in k-consumption order (per k the
                        # stream needs wg then wd) and smallest first, so
                        # the first matmuls wait on as little as possible
                        for c0, c1 in ((0, 2), (2, 4), (4, 10), (10, 16)):
                            cols = slice(c0 * P, c1 * P)
                            nc.sync.dma_start(out=wgp[:, cols],
                                              in_=wg.ap()[0][:, cols])
                            nc.sync.dma_start(out=wdp[:, cols],
                                              in_=wd.ap()[0][:, cols])
                        # x supply, ramped piece sizes, each piece one 2D
                        # transfer, per-ring strictly in need-order (the
                        # ~4 semaphore slots per ring gate dispatch on the
                        # completion of the slot's previous transfer, so a
                        # big early transfer delays everything behind it).
                        # Scalar: all of strip 0, then strips 2/3b.
                        xpiece(nc.scalar, 0, 0, 1)     # k0      128KB
                        xpiece(nc.scalar, 0, 1, 2)     # k1      128KB
                        xpiece(nc.scalar, 0, 2, 4)     # k2-3    256KB
                        xpiece(nc.scalar, 0, 4, 6)     # k4-5    256KB
                        xpiece(nc.scalar, 0, 6, 8)     # k6-7    256KB
                        xpiece(nc.scalar, 0, 8, 12)    # k8-11   512KB
                        xpiece(nc.scalar, 0, 12, 16)   # k12-15  512KB
                        # Sync (behind the j0 panels): strip 1 quads; its
                        # last quad rides Scalar (sync runs ~1us behind)
                        for q in range(3):
                            xpiece(nc.sync, 1, 4 * q, 4 * q + 4)
                        xpiece(nc.scalar, 1, 12, 16)
                        # remaining strips in need-order on both rings
                        xpiece(nc.scalar, 2, 0, 8)     # strip2a   1MB
                        xpiece(nc.scalar, 2, 8, 16)    # strip2b   1MB
                        xpiece(nc.sync, 3, 0, 4)       # strip3 quads
                        xpiece(nc.sync, 3, 4, 8)
                        xpiece(nc.scalar, 3, 8, 12)
                        xpiece(nc.scalar, 3, 12, 16)
                    elif j == 1:
                        # j1 panels on the Sync ring behind the strips
                        # (they are not needed before ~38us)
                        nc.sync.dma_start(out=wgp, in_=wg.ap()[j])
                        nc.sync.dma_start(out=wdp, in_=wd.ap()[j])
                    else:
                        nc.sync.dma_start(out=wgp, in_=wg.ap()[j])
                        nc.sync.dma_start(out=wdp, in_=wd.ap()[j])
                    for s in range(S1):
                        pg = p1.tile([P, NS], F32, tag="pg", name="pg")
                        pu = p1.tile([P, NS], F32, tag="pu", name="pu")
                        # interleave gate/down per k: halves the x-chunk
                        # consumption rate so DMA supply keeps up during j0
                        for k in range(KC):
                            nc.tensor.matmul(
                                pg, lhsT=wgp[:, k * P:(k + 1) * P],
                                rhs=x_rhs(s, k),
                                start=(k == 0), stop=(k == KC - 1))
                            nc.tensor.matmul(
                                pu, lhsT=wdp[:, k * P:(k + 1) * P],
                                rhs=x_rhs(s, k),
                                start=(k == 0), stop=(k == KC - 1))
                        sl = silu_pool.tile([P, NS], BF16, tag="sl", name="sl")
                        if sim_safe:
                            # CoreSim has no Silu; silu(g) = g * sigmoid(g)
                            nc.scalar.activation(sl, pg, SIGMOID)
                            nc.vector.tensor_mul(out=sl, in0=sl, in1=pg)
                        else:
                            nc.scalar.activation(sl, pg, SILU)
                        nc.vector.tensor_mul(out=hT[j][:, s * NS:(s + 1) * NS],
                                             in0=sl, in1=pu)

                # stage phase-2 weights; the Sync ring reaches these right
                # after the phase-1 panels, well before phase 2 needs them
                for j in range(JC):
                    nc.sync.dma_start(out=wub[j], in_=wu.ap()[j])

                # ---- phase 2: out = hT.T @ Wu ----
                for m in range(MC):
                    for n in range(S2):
                        msl = slice(m * P, (m + 1) * P)
                        if m == MC - 1 and n == S2 - 1:
                            # final group as two N=256 halves: the first
                            # half's evict+DMA overlaps the second half's
                            # matmuls, shortening the kernel tail
                            H = NS // 2
                            for h in range(2):
                                dsl = slice(n * NS + h * H,
                                            n * NS + (h + 1) * H)
                                po = p2.tile([P, H], F32, tag="po",
                                             name=f"poL{h}")
                                for j in range(JC):
                                    nc.tensor.matmul(
                                        po, lhsT=hT[j][:, msl],
                                        rhs=wub[j][:, dsl],
                                        start=(j == 0), stop=(j == JC - 1))
                                ot = ostage.tile([P, H], BF16, tag="ot",
                                                 name=f"oL{h}")
                                # both halves ride the Scalar ring: the
                                # Sync queue carries ~125KB of out-DMA
                                # backlog at this point, and engines
                                # round-robin between queues per packet,
                                # so the clean queue bypasses it
                                if h == 0:
                                    nc.vector.tensor_copy(out=ot, in_=po)
                                    nc.sync.dma_start(
                                        out=out.ap()[msl, dsl], in_=ot)
                                else:
                                    nc.scalar.activation(ot, po, COPY)
                                    nc.scalar.dma_start(
                                        out=out.ap()[msl, dsl], in_=ot)
                            continue
                        dsl = slice(n * NS, (n + 1) * NS)
                        po = p2.tile([P, NS], F32, tag="po", name="po")
                        for j in range(JC):
                            nc.tensor.matmul(
                                po, lhsT=hT[j][:, msl],
                                rhs=wub[j][:, dsl],
                                start=(j == 0), stop=(j == JC - 1))
                        ot = ostage.tile([P, NS], BF16, tag="ot", name="ot")
                        if (m * S2 + n) % 2 == 0:
                            nc.scalar.activation(ot, po, COPY)
                        else:
                            nc.vector.tensor_copy(out=ot, in_=po)
                        # last row's blocks also bypass the Sync backlog
                        ring = nc.scalar if m == MC - 1 else nc.sync
                        ring.dma_start(
                            out=out.ap()[msl, dsl], in_=ot)

    nc.finalize()
    return nc


_BF = ml_dtypes.bfloat16


def make_in_maps(x, gate_proj, down_proj, up_proj):
    maps = []
    for e in range(E):
        xtb = x[e].T.astype(_BF)  # [DIN, T]
        # [S1, P, KC*NS]: per-(s,p) all 16 k-chunk rows are one contiguous
        # 16KB run, so whole strips move as single 2D transfers
        xtb = np.ascontiguousarray(
            xtb.reshape(KC, P, S1, NS).transpose(2, 1, 0, 3)
        ).reshape(S1, P, KC * NS)
        gtb = np.ascontiguousarray(
            gate_proj[e].astype(_BF).reshape(KC, P, JC, P)
            .transpose(2, 1, 0, 3)).reshape(JC, P, DIN)
        dtb = np.ascontiguousarray(
            down_proj[e].astype(_BF).reshape(KC, P, JC, P)
            .transpose(2, 1, 0, 3)).reshape(JC, P, DIN)
        utb = np.ascontiguousarray(up_proj[e].astype(_BF)).reshape(JC, P, DIN)
        maps.append({"x_t": xtb, "gate_t": gtb, "down_t": dtb, "up_t": utb})
    return maps


_program = None


def kernel(x, gate_proj, down_proj, up_proj):
    global _program
    if _program is None:
        _program = build_program()
    in_maps = make_in_maps(
        np.asarray(x, dtype=np.float32),
        np.asarray(gate_proj, dtype=np.float32),
        np.asarray(down_proj, dtype=np.float32),
        np.asarray(up_proj, dtype=np.float32),
    )
    res = run_bass_kernel_spmd(_program, in_maps, list(range(E)))
    return np.stack(
        [np.asarray(res.results[e]["out"]).astype(np.float32)
         for e in range(E)], axis=0)



# revision 44
# speedup vs baseline: 1.0059x; 1.0059x over previous
"""Grouped-expert SwiGLU (MoE) kernel for Trainium2, expert-parallel over 8 cores.

Per core (one expert):
    g = x @ W_gate          [T, DOUT]
    u = x @ W_down          [T, DOUT]
    h = silu(g) * u
    out = h @ W_up          [T, DIN]

All inputs are pre-cast to bf16 and pre-laid-out on the host so the device
does no transposes and no input casts — the PE runs a dense LDW+MM stream at
the bf16 roofline (~216 ns per [128x128]x[128x512] matmul):
  x_t    [S1, P, KC*NS]   strip-contiguous xT: x_t[s,p,k*NS+n] = x[s*NS+n, k*P+p]
  gate_t [JC, P, DIN]     per-j panels: gate_t[j,p,k*P+n] = Wg[k*P+p, j*P+n]
  down_t [JC, P, DIN]     same layout as gate_t
  up_t   [JC, P, DIN]     up_t[j,p,c] = Wu[j*P+p, c]
phase 1: hT[j] = silu(Wg[:,j].T @ xT) * (Wd[:,j].T @ xT)   [dout, tokens]
phase 2: out[m,:] = sum_j hT[j][:,m].T @ Wu[j,:]           [tokens, din]
Matmuls in bf16 with fp32 PSUM accumulation; out stored bf16 (rel err
~0.0047 vs the fp32 reference) and upcast to fp32 on the host.

DMA notes (measured): each DMA_DIRECT2D dispatch costs ~650ns serialized on
its ring (Sync/Scalar are the two HWDGE rings); a single in-flight transfer
streams ~100GB/s, concurrent transfers ~400GB/s aggregate; the tile
framework cycles ~10 global in-flight DMA semaphore slots, with each
dispatch gated on the completion of its slot's previous transfer.  So the
j0 window emits DMAs in exact consumption order, finest pieces first,
across both rings; later strips use big transfers (fewer dispatches).  The
PE clock ramps 0.65->1.2->2.4GHz over ~3-6us of continuous execution and
drops back on multi-us idle gaps, so WARMUP_MM full-width dummy matmuls
ramp it while the first DMAs are in flight.
"""

import numpy as np
import ml_dtypes

import concourse.bacc as bacc
import concourse.mybir as mybir
from concourse.tile import TileContext
from concourse.bass_utils import run_bass_kernel_spmd

F32 = mybir.dt.float32
BF16 = mybir.dt.bfloat16
SILU = mybir.ActivationFunctionType.Silu
SIGMOID = mybir.ActivationFunctionType.Sigmoid
COPY = mybir.ActivationFunctionType.Copy

E = 8
T, DIN, DOUT = 2048, 2048, 1408
P = 128
NS = 512
KC = DIN // P   # 16 contraction chunks (din)
JC = DOUT // P  # 11 dout blocks
MC = T // P     # 16 token blocks
S1 = T // NS    # 4 token strips
S2 = DIN // NS  # 4 din strips

WARMUP_MM = 9   # dummy matmuls to ramp the PE clock before real data lands
WARMUP_N = 512  # full-width: narrow warmups are LDW-bound (PE array half
                # idle between them) and the DVFS ramp stalls at mid clock


def build_program(sim_safe=False):
    nc = bacc.Bacc(target_bir_lowering=False, trn_type="TRN2")
    # strip-contiguous x layout: x_t[s, p, k*NS+n] = x[s*NS+n, k*P+p], so a
    # whole strip is ONE 2D transfer ([128 x 16KB] = 2MB, 16KB descriptors)
    # and any k-range piece is still contiguous.  Bigger transfers matter
    # because the tile framework cycles only ~10 global in-flight DMA
    # semaphore slots — each slot must carry as many bytes as possible.
    xt = nc.dram_tensor("x_t", [S1, P, KC * NS], BF16,
                        kind="ExternalInput")
    wg = nc.dram_tensor("gate_t", [JC, P, DIN], BF16, kind="ExternalInput")
    wd = nc.dram_tensor("down_t", [JC, P, DIN], BF16, kind="ExternalInput")
    wu = nc.dram_tensor("up_t", [JC, P, DIN], BF16, kind="ExternalInput")
    # out stored bf16 (upcast on host): halves out-DMA bytes and the tail
    out = nc.dram_tensor("out", [T, DIN], BF16, kind="ExternalOutput")

    with TileContext(nc) as tc:
        with tc.tile_pool(name="persist", bufs=1) as persist:
            xts = [persist.tile([P, KC, NS], BF16, tag=f"xts{s}",
                                name=f"xts{s}")
                   for s in range(S1)]
            hT = [persist.tile([P, T], BF16, tag=f"hT{j}", name=f"hT{j}")
                  for j in range(JC)]
            wub = [persist.tile([P, DIN], BF16, tag=f"wub{j}", name=f"wub{j}")
                   for j in range(JC)]
            scratch = persist.tile([P, NS], BF16, tag="scratch",
                                   name="scratch")

            with tc.tile_pool(name="wstage", bufs=2) as wstage, \
                 tc.tile_pool(name="silu", bufs=3) as silu_pool, \
                 tc.tile_pool(name="ostage", bufs=6) as ostage, \
                 tc.tile_pool(name="p1", bufs=2, space="PSUM") as p1, \
                 tc.tile_pool(name="p2", bufs=4, space="PSUM") as p2:

                def x_rhs(s, k):
                    return xts[s][:, k, :]

                def xpiece(ring, s, k0, k1):
                    # k-chunks [k0, k1) of strip s as one 2D transfer
                    ring.dma_start(
                        out=xts[s][:, k0:k1, :],
                        in_=xt.ap()[s][:, k0 * NS:k1 * NS]
                        .rearrange("p (k n) -> p k n", n=NS))

                # PE clock warmup: the PE clock ramps 0.65->1.2->2.4GHz
                # over ~3us of continuous execution, so dummy matmuls keep
                # it busy (and ramping) from the end of the engine barrier
                # (~7us) until real data lands (~10.3us); the first real
                # matmul then runs at full clock.  GpSimd memsets scratch —
                # its queue is free right after the framework's own const
                # memsets, the earliest any engine can write SBUF.  The
                # products are discarded (PSUM slot borrowed from p2's
                # rotation, overwritten with start=True in phase 2).
                nc.gpsimd.memset(scratch[:, :], 0.0)
                pw = p2.tile([P, NS], F32, tag="po", name="pw")
                for w in range(WARMUP_MM):
                    nc.tensor.matmul(pw[:, :WARMUP_N],
                                     lhsT=scratch[:, :P],
                                     rhs=scratch[:, :WARMUP_N],
                                     start=True, stop=True)

                # ---- phase 1: hT[j] = silu(gT) * uT ----
                for j in range(JC):
                    wgp = wstage.tile([P, DIN], BF16, tag="wgp", name=f"wgp{j}")
                    wdp = wstage.tile([P, DIN], BF16, tag="wdp", name=f"wdp{j}")
                    if j == 0:
                        # j0 panels in pieces on the Sync ring, wg/wd
                        # interleaved in k-consumption order (per k the
                        # stream needs wg then wd) and smallest first, so
                        # the first matmuls wait on as little as possible
                        for c0, c1 in ((0, 2), (2, 4), (4, 10), (10, 16)):
                            cols = slice(c0 * P, c1 * P)
                            nc.sync.dma_start(out=wgp[:, cols],
                                              in_=wg.ap()[0][:, cols])
                            nc.sync.dma_start(out=wdp[:, cols],
                                              in_=wd.ap()[0][:, cols])
                        # x supply, ramped piece sizes, each piece one 2D
                        # transfer, per-ring strictly in need-order (the
                        # ~4 semaphore slots per ring gate dispatch on the
                        # completion of the slot's previous transfer, so a
                        # big early transfer delays everything behind it).
                        # Scalar: all of strip 0, then strips 2/3b.
                        xpiece(nc.scalar, 0, 0, 1)     # k0      128KB
                        xpiece(nc.scalar, 0, 1, 2)     # k1      128KB
                        xpiece(nc.scalar, 0, 2, 4)     # k2-3    256KB
                        xpiece(nc.scalar, 0, 4, 6)     # k4-5    256KB
                        xpiece(nc.scalar, 0, 6, 8)     # k6-7    256KB
                        xpiece(nc.scalar, 0, 8, 12)    # k8-11   512KB
                        xpiece(nc.scalar, 0, 12, 16)   # k12-15  512KB
                        # Sync (behind the j0 panels): strip 1 quads; its
                        # last quad rides Scalar (sync runs ~1us behind)
                        for q in range(3):
                            xpiece(nc.sync, 1, 4 * q, 4 * q + 4)
                        xpiece(nc.scalar, 1, 12, 16)
                        # remaining strips in need-order on both rings
                        xpiece(nc.scalar, 2, 0, 8)     # strip2a   1MB
                        xpiece(nc.scalar, 2, 8, 16)    # strip2b   1MB
                        xpiece(nc.sync, 3, 0, 4)       # strip3 quads
                        xpiece(nc.sync, 3, 4, 8)
                        xpiece(nc.scalar, 3, 8, 12)
                        xpiece(nc.scalar, 3, 12, 16)
                    elif j == 1:
                        # j1 panels on the Sync ring behind the strips
                        # (they are not needed before ~38us)
                        nc.sync.dma_start(out=wgp, in_=wg.ap()[j])
                        nc.sync.dma_start(out=wdp, in_=wd.ap()[j])
                    else:
                        nc.sync.dma_start(out=wgp, in_=wg.ap()[j])
                        nc.sync.dma_start(out=wdp, in_=wd.ap()[j])
                    for s in range(S1):
                        pg = p1.tile([P, NS], F32, tag="pg", name="pg")
                        pu = p1.tile([P, NS], F32, tag="pu", name="pu")
                        # interleave gate/down per k: halves the x-chunk
                        # consumption rate so DMA supply keeps up during j0
                        for k in range(KC):
                            nc.tensor.matmul(
                                pg, lhsT=wgp[:, k * P:(k + 1) * P],
                                rhs=x_rhs(s, k),
                                start=(k == 0), stop=(k == KC - 1))
                            nc.tensor.matmul(
                                pu, lhsT=wdp[:, k * P:(k + 1) * P],
                                rhs=x_rhs(s, k),
                                start=(k == 0), stop=(k == KC - 1))
                        sl = silu_pool.tile([P, NS], BF16, tag="sl", name="sl")
                        if sim_safe:
                            # CoreSim has no Silu; silu(g) = g * sigmoid(g)
                            nc.scalar.activation(sl, pg, SIGMOID)
                            nc.vector.tensor_mul(out=sl, in0=sl, in1=pg)
                        else:
                            nc.scalar.activation(sl, pg, SILU)
                        nc.vector.tensor_mul(out=hT[j][:, s * NS:(s + 1) * NS],
                                             in0=sl, in1=pu)

                # stage phase-2 weights; the Sync ring reaches these right
                # after the phase-1 panels, well before phase 2 needs them
                for j in range(JC):
                    nc.sync.dma_start(out=wub[j], in_=wu.ap()[j])

                # ---- phase 2: out = hT.T @ Wu ----
                for m in range(MC):
                    for n in range(S2):
                        msl = slice(m * P, (m + 1) * P)
                        if m == MC - 1 and n == S2 - 1:
                            # final group as two N=256 halves: the first
                            # half's evict+DMA overlaps the second half's
                            # matmuls, shortening the kernel tail
                            H = NS // 2
                            for h in range(2):
                                dsl = slice(n * NS + h * H,
                                            n * NS + (h + 1) * H)
                                po = p2.tile([P, H], F32, tag="po",
                                             name=f"poL{h}")
                                for j in range(JC):
                                    nc.tensor.matmul(
                                        po, lhsT=hT[j][:, msl],
                                        rhs=wub[j][:, dsl],
                                        start=(j == 0), stop=(j == JC - 1))
                                ot = ostage.tile([P, H], BF16, tag="ot",
                                                 name=f"oL{h}")
                                # both halves ride the Scalar ring: the
                                # Sync queue carries ~125KB of out-DMA
                                # backlog at this point, and engines
                                # round-robin between queues per packet,
                                # so the clean queue bypasses it
                                if h == 0:
                                    nc.vector.tensor_copy(out=ot, in_=po)
                                else:
                                    nc.scalar.activation(ot, po, COPY)
                                nc.scalar.dma_start(
                                    out=out.ap()[msl, dsl], in_=ot)
                            continue
                        dsl = slice(n * NS, (n + 1) * NS)
                        po = p2.tile([P, NS], F32, tag="po", name="po")
                        for j in range(JC):
                            nc.tensor.matmul(
                                po, lhsT=hT[j][:, msl],
                                rhs=wub[j][:, dsl],
                                start=(j == 0), stop=(j == JC - 1))
                        ot = ostage.tile([P, NS], BF16, tag="ot", name="ot")
                        if (m * S2 + n) % 2 == 0:
                            nc.scalar.activation(ot, po, COPY)
                        else:
                            nc.vector.tensor_copy(out=ot, in_=po)
                        # last row's blocks also bypass the Sync backlog
                        ring = nc.scalar if m == MC - 1 else nc.sync
                        ring.dma_start(
                            out=out.ap()[msl, dsl], in_=ot)

    nc.finalize()
    return nc


_BF = ml_dtypes.bfloat16


def make_in_maps(x, gate_proj, down_proj, up_proj):
    maps = []
    for e in range(E):
        xtb = x[e].T.astype(_BF)  # [DIN, T]
        # [S1, P, KC*NS]: per-(s,p) all 16 k-chunk rows are one contiguous
        # 16KB run, so whole strips move as single 2D transfers
        xtb = np.ascontiguousarray(
            xtb.reshape(KC, P, S1, NS).transpose(2, 1, 0, 3)
        ).reshape(S1, P, KC * NS)
        gtb = np.ascontiguousarray(
            gate_proj[e].astype(_BF).reshape(KC, P, JC, P)
            .transpose(2, 1, 0, 3)).reshape(JC, P, DIN)
        dtb = np.ascontiguousarray(
            down_proj[e].astype(_BF).reshape(KC, P, JC, P)
            .transpose(2, 1, 0, 3)).reshape(JC, P, DIN)
        utb = np.ascontiguousarray(up_proj[e].astype(_BF)).reshape(JC, P, DIN)
        maps.append({"x_t": xtb, "gate_t": gtb, "down_t": dtb, "up_t": utb})
    return maps


_program = None


def kernel(x, gate_proj, down_proj, up_proj):
    global _program
    if _program is None:
        _program = build_program()
    in_maps = make_in_maps(
        np.asarray(x, dtype=np.float32),
        np.asarray(gate_proj, dtype=np.float32),
        np.asarray(down_proj, dtype=np.float32),
        np.asarray(up_proj, dtype=np.float32),
    )
    res = run_bass_kernel_spmd(_program, in_maps, list(range(E)))
    return np.stack(
        [np.asarray(res.results[e]["out"]).astype(np.float32)
         for e in range(E)], axis=0)

